# revision 28
# baseline (speedup 1.0000x reference)
"""Trainium2 Bass kernel for nn_AttentionHead (B=16, T=2048, DIM=512, HEAD=64).

Strategy: data-parallel over batch across 8 NeuronCores (2 batches/core).
Host-side prep (free): x is pre-transposed to [DIM, T] bf16 per batch, the
Wq/Wk projection weights are stacked so one matmul produces [Q^T; K^T], and
the rotary coefficient tables ship as one [128, T] f16 payload expanded
on-device. The pair-swap for rotary is a permutation matmul on-device.

Schedule: input loading is HBM-bound (~25us for 8MB), so each batch's
prologue is chained per 512-wide tile (proj -> psum copy -> pair-swap ->
rotary -> V transposes) and the first attention block for b0 is emitted
BEFORE b1's prologue — the ScalarE exp stream (the steady-state bottleneck,
~1.1us per [128,1024] tile) starts as soon as tile 0 of b0 is through.
b1's psum copies run on the DVE so the in-order ScalarE stream can't stall
the exps behind them. Per-batch psum rings (tag po<b>) keep the two
batches' pipelines independent; everything else is balanced across engines
(input DMAs on GpSimd+Sync queues, consts on the ScalarE HWDGE queue,
rotary dups on GpSimd, output scale-copies on DVE).

Per-core graph (per batch):
  QK^T = Wqk^T @ x^T (PE, bf16)            -> [128, T] psum
  swap = P_pairswap @ QK^T (PE, f16)
  q~/k~ = QK^T*fr + swap*fi (DVE, f16)     -> rotary applied, [64, T] each
  S^T[j-chunk] = k~[j].T @ q~ (PE, f16)    -> [128, 1024] psum per chunk
  P^T = exp(S^T / sqrt(512)) (ACT, fp16)   -> no max-subtraction needed:
        |S|/sqrt(512) <= ~5 so exp is safely bounded in fp16/f32
  out^T += V~[j].T @ P^T (PE, fp16)        -> V~ has a ones column so row 64
        accumulates the softmax denominator
  out^T * 2^-6 -> f16 (DVE), DMA to DRAM [65, T]; host divides row 64 and
        transposes back to [T, 64] f32.
"""

import os
import sys

for _p in ("/opt/trn_rl_repo", "/root/.axon_site/_ro/trn_rl_repo"):
    if os.path.isdir(_p) and _p not in sys.path:
        sys.path.append(_p)

import numpy as np
import ml_dtypes

import concourse.bass as bass
import concourse.mybir as mybir
import concourse.tile as tile
from concourse import bacc
from concourse.bass import ts
from concourse.bass_utils import run_bass_kernel_spmd

F32 = mybir.dt.float32
F32R = mybir.dt.float32r
BF16 = mybir.dt.bfloat16
F16 = mybir.dt.float16

B, T, DIM, HEAD = 16, 2048, 512, 64
NCORES = 8
BPC = B // NCORES          # batches per core
NCC = DIM // 128           # contraction chunks
NT = T // 512              # 512-wide tiles along t
NJ = T // 128              # key chunks
IH = T // 512              # query quarter blocks
IW = 512                   # query block width
OSCALE = 1.0 / 64.0        # keeps out^T accumulators in f16 range


def _build():
    scale = 1.0 / np.sqrt(np.float32(DIM))
    nc = bacc.Bacc(None, target_bir_lowering=False)
    xt_e = nc.declare_dram_parameter("xt", [BPC, DIM, T], BF16, isOutput=False)
    w_e = nc.declare_dram_parameter("w", [128, NCC * (128 + HEAD)], BF16,
                                    isOutput=False)
    frfi_e = nc.declare_dram_parameter("frfi", [128, T], F16, isOutput=False)
    id_e = nc.declare_dram_parameter("ident", [128, 128], F16, isOutput=False)
    out_e = nc.declare_dram_parameter("out", [BPC, HEAD + 1, T], F16,
                                      isOutput=True)

    with tile.TileContext(nc) as tc:
        with (
            tc.tile_pool(name="consts", bufs=1) as cp,
            tc.tile_pool(name="xt", bufs=1) as xp,
            tc.tile_pool(name="big", bufs=1) as bp,
            tc.tile_pool(name="vtp", bufs=1) as vp,
            tc.tile_pool(name="ptp", bufs=5) as pp,
            tc.tile_pool(name="op", bufs=3) as op,
            tc.tile_pool(name="psS", bufs=2, space="PSUM") as psS,
            tc.tile_pool(name="psO", bufs=2, space="PSUM") as psO,
        ):
            # Consts on the ScalarE HWDGE queue (empty at start) so they land
            # before the xt bulk transfers saturate the other two queues.
            w_all = cp.tile([128, NCC * (128 + HEAD)], BF16, tag="w")
            nc.scalar.dma_start(out=w_all, in_=w_e[:])
            wqk_t = [w_all[:, ci * 192:ci * 192 + 128] for ci in range(NCC)]
            wv_t = [w_all[:, ci * 192 + 128:ci * 192 + 192]
                    for ci in range(NCC)]
            frfi_t = cp.tile([128, T], F16, tag="frfi")
            nc.scalar.dma_start(out=frfi_t, in_=frfi_e[:])
            id16 = cp.tile([128, 128], F16, tag="id16")
            nc.scalar.dma_start(out=id16, in_=id_e[:])

            # Input DMAs: [128, T/2] half-chunks, b0 first, low halves of
            # all chunks before high halves, queues alternated — the first
            # projection starts after ~2MB instead of ~4MB, and b1's data
            # finishes sooner.
            HT = T // 2
            xts_b = []
            for b in range(BPC):
                halves = [[], []]
                for h in range(2):
                    for ci in range(NCC):
                        xh = xp.tile([128, HT], BF16, tag=f"xt{b}_{ci}_{h}")
                        eng = nc.gpsimd if ci % 2 == 0 else nc.sync
                        eng.dma_start(out=xh,
                                      in_=xt_e[b, ts(ci, 128), ts(h, HT)])
                        halves[h].append(xh)
                xts_b.append(halves)

            # fr stays f16 (multiplied with f16 qk), fi is f32 (multiplied
            # with the f32 psum swap output).
            fr_t = cp.tile([128, T], F16, tag="fr")
            nc.vector.tensor_copy(fr_t[0:64, :], frfi_t[0:64, :])
            nc.vector.tensor_copy(fr_t[64:128, :], frfi_t[0:64, :])
            fi_t = cp.tile([128, T], F16, tag="fi")
            nc.vector.tensor_copy(fi_t[0:64, :], frfi_t[64:128, :])
            nc.vector.tensor_copy(fi_t[64:128, :], frfi_t[64:128, :])

            per_b = [None] * BPC
            vts_b = [None] * BPC

            state_b = [None] * BPC

            def prologue_start(b):
                halves = xts_b[b]
                qk_s = bp.tile([128, T], F16, tag=f"qk{b}")
                vT_s = bp.tile([HEAD, T], F16, tag=f"vT{b}")
                qd = bp.tile([128, T], F16, tag=f"qd{b}")
                kd = bp.tile([128, T], F16, tag=f"kd{b}")
                t1 = bp.tile([128, T], F16, tag=f"t1{b}")
                t2 = bp.tile([128, T], F16, tag=f"t2{b}")
                vts = []
                # b0's psum copies go on ScalarE (idle before the exps);
                # b1's go on the DVE so the in-order ScalarE stream can't
                # block b0's exps behind b1's late psums.
                if b == 0:
                    copy = nc.scalar.copy
                else:
                    def copy(out, in_):
                        nc.vector.tensor_copy(out, in_)
                sw_s = bp.tile([128, T], F16, tag=f"sw{b}")
                swq = nc.sync if b == 0 else nc.gpsimd
                per_b[b] = (qd, kd)
                vts_b[b] = vts
                state_b[b] = (halves, qk_s, vT_s, qd, kd, t1, t2, vts, copy,
                              sw_s, swq)

            def prologue_tile(b, tt):
                """Emit one 512-wide tile's chain: proj -> copies -> swap ->
                rotary -> V transposes. Tile tt is attention-ready for key
                pairs 2tt/2tt+1 when it completes."""
                (halves, qk_s, vT_s, qd, kd, t1, t2, vts, copy,
                 sw_s, swq) = state_b[b]
                if True:
                    sl = ts(tt, 512)
                    xs = halves[tt // 2]
                    xsl = ts(tt % 2, 512)
                    pqk = psS.tile([128, 512], F32, tag="s")
                    for ci in range(NCC):
                        nc.tensor.matmul(pqk, wqk_t[ci], xs[ci][:, xsl],
                                         start=(ci == 0), stop=(ci == NCC - 1))
                    copy(out=qk_s[:, sl], in_=pqk)
                    pv = psS.tile([HEAD, 512], F32, tag="s")
                    for ci in range(NCC):
                        nc.tensor.matmul(pv, wv_t[ci], xs[ci][:, xsl],
                                         start=(ci == 0), stop=(ci == NCC - 1))
                    copy(out=vT_s[:, sl], in_=pv)

                    # Pair-swap via partition-strided SBUF-to-SBUF DMAs
                    # (replaces a PE permutation matmul).
                    swq.dma_start(out=sw_s[0:127:2, sl], in_=qk_s[1:128:2, sl])
                    swq.dma_start(out=sw_s[1:128:2, sl], in_=qk_s[0:127:2, sl])
                    nc.vector.tensor_tensor(out=t1[:, sl], in0=qk_s[:, sl],
                                            in1=fr_t[:, sl],
                                            op=mybir.AluOpType.mult)
                    nc.vector.tensor_tensor(out=t2[:, sl], in0=sw_s[:, sl],
                                            in1=fi_t[:, sl],
                                            op=mybir.AluOpType.mult)
                    nc.vector.tensor_tensor(out=qd[0:64, sl],
                                            in0=t1[0:64, sl],
                                            in1=t2[0:64, sl],
                                            op=mybir.AluOpType.add)
                    dupq = nc.gpsimd if b == 0 else nc.sync
                    dupq.dma_start(out=qd[64:128, sl], in_=qd[0:64, sl])
                    nc.vector.tensor_tensor(out=kd[64:128, sl],
                                            in0=t1[64:128, sl],
                                            in1=t2[64:128, sl],
                                            op=mybir.AluOpType.add)
                    dupq.dma_start(out=kd[0:64, sl], in_=kd[64:128, sl])

                    for j in range(4 * tt, 4 * tt + 4):
                        pvt = psS.tile([128, HEAD], F16, tag="s")
                        nc.tensor.transpose(pvt, vT_s[:, ts(j, 128)],
                                            id16[0:HEAD, 0:HEAD])
                        vt = vp.tile([128, HEAD + 1], F16, tag=f"vt{b}_{j}")
                        nc.vector.tensor_copy(vt[:, 0:HEAD], pvt)
                        nc.vector.memset(vt[:, HEAD:HEAD + 1], 1.0)
                        vts.append(vt)

            NP = NJ // 2

            def attn(ih, jp, b, po_t):
                jA, jB = 2 * jp, 2 * jp + 1
                qd, kd = per_b[b]
                vts = vts_b[b]
                sp = psS.tile([128, 1024], F32, tag="s")
                nc.tensor.matmul(sp[:, 0:512],
                                 kd[0:64, ts(jA, 128)],
                                 qd[0:64, ts(ih, 512)],
                                 start=True, stop=True)
                nc.tensor.matmul(sp[:, 512:1024],
                                 kd[64:128, ts(jB, 128)],
                                 qd[64:128, ts(ih, 512)],
                                 start=True, stop=True)
                pT = pp.tile([128, 1024], F16, tag="pT")
                nc.scalar.activation(out=pT, in_=sp,
                                     func=mybir.ActivationFunctionType.Exp,
                                     scale=float(scale))
                nc.tensor.matmul(po_t, vts[jA], pT[:, 0:512],
                                 start=(jp == 0), stop=False,
                                 skip_group_check=True)
                nc.tensor.matmul(po_t, vts[jB], pT[:, 512:1024],
                                 start=False, stop=(jp == NP - 1),
                                 skip_group_check=True)

            def emit_out(ih, b, po_t):
                oc = op.tile([HEAD + 1, IW], F16, tag="oc")
                nc.vector.tensor_scalar_mul(out=oc, in0=po_t,
                                            scalar1=float(OSCALE))
                nc.sync.dma_start(out=out_e[b, :, ts(ih, IW)], in_=oc)

            # b0's prologue tiles interleave with its ih=0 attention: the
            # exp stream starts as soon as tile 0 is through. b0's ih=1
            # block follows (ScalarE backlog) before b1's prologue, which
            # interleaves with b1's ih=0 the same way. The remaining blocks
            # pair up across batches to keep the exp stream saturated.
            prologue_start(0)
            po00 = psO.tile([HEAD + 1, IW], F32, tag="po0")
            for tt in range(NT):
                prologue_tile(0, tt)
                attn(0, 2 * tt, 0, po00)
                attn(0, 2 * tt + 1, 0, po00)
            emit_out(0, 0, po00)

            po10 = psO.tile([HEAD + 1, IW], F32, tag="po0")
            for jp in range(NP):
                attn(1, jp, 0, po10)
            emit_out(1, 0, po10)

            prologue_start(1)
            po01 = psO.tile([HEAD + 1, IW], F32, tag="po1")
            for tt in range(NT):
                prologue_tile(1, tt)
                attn(0, 2 * tt, 1, po01)
                attn(0, 2 * tt + 1, 1, po01)
            emit_out(0, 1, po01)

            for pair in ([(2, 0), (1, 1)], [(3, 0), (2, 1)], [(3, 1)]):
                pos = []
                for ih, b in pair:
                    po_t = psO.tile([HEAD + 1, IW], F32, tag=f"po{b}")
                    pos.append(po_t)
                for jp in range(NP):
                    for (ih, b), po_t in zip(pair, pos):
                        attn(ih, jp, b, po_t)
                for (ih, b), po_t in zip(pair, pos):
                    emit_out(ih, b, po_t)
    nc.compile()
    return nc


def _prep_consts(Wq, Wk, Wv, fx_real, fx_imag, fy_real, fy_imag):
    WqT = np.asarray(Wq, np.float32).T
    WkT = np.asarray(Wk, np.float32).T
    WvT = np.asarray(Wv, np.float32).T
    wqk = np.concatenate([WqT, WkT], axis=1).reshape(NCC, 128, 128)
    wv = WvT.reshape(NCC, 128, HEAD)
    w = np.concatenate([wqk, wv], axis=2)          # [NCC, 128, 192]
    w = np.ascontiguousarray(w.transpose(1, 0, 2).reshape(128, -1)).astype(
        ml_dtypes.bfloat16)

    fx_real = np.asarray(fx_real, np.float32)
    fx_imag = np.asarray(fx_imag, np.float32)
    fy_real = np.asarray(fy_real, np.float32)
    fy_imag = np.asarray(fy_imag, np.float32)
    fr64 = np.zeros((64, T), np.float32)
    fi64 = np.zeros((64, T), np.float32)
    for h in range(64):
        if h < 32:
            frs, fis, p = fx_real, fx_imag, h // 2
        else:
            frs, fis, p = fy_real, fy_imag, (h - 32) // 2
        fr64[h] = frs[:, p]
        fi64[h] = fis[:, p] * (-1.0 if h % 2 == 0 else 1.0)
    frfi = np.concatenate([fr64, fi64], axis=0).astype(np.float16)
    ident = np.eye(128, dtype=np.float16)
    return dict(w=w, frfi=frfi, ident=ident)


_NC_CACHE = {}


def _get_nc():
    if "nc" not in _NC_CACHE:
        _NC_CACHE["nc"] = _build()
    return _NC_CACHE["nc"]


def kernel(x, Wq, Wk, Wv, fx_real, fx_imag, fy_real, fy_imag):
    x = np.asarray(x, np.float32)
    xt = np.ascontiguousarray(x.transpose(0, 2, 1)).astype(ml_dtypes.bfloat16)
    consts = _prep_consts(Wq, Wk, Wv, fx_real, fx_imag, fy_real, fy_imag)
    in_maps = []
    for c in range(NCORES):
        m = {"xt": xt[c * BPC:(c + 1) * BPC]}
        m.update(consts)
        in_maps.append(m)
    nc = _get_nc()
    res = run_bass_kernel_spmd(nc, in_maps, core_ids=list(range(NCORES)))
    # res: per-core [BPC, 65, T] f16 -> divide by denominator row, transpose
    outs = []
    for c in range(NCORES):
        o = np.asarray(res.results[c]["out"], np.float32)
        outs.append((o[:, 0:HEAD, :] / o[:, HEAD:HEAD + 1, :]).transpose(0, 2, 1))
    return np.ascontiguousarray(np.concatenate(outs, axis=0))


# revision 29
# speedup vs baseline: 1.5497x; 1.5497x over previous
"""Trainium2 Bass kernel for nn_AttentionHead (B=16, T=2048, DIM=512, HEAD=64).

Strategy: data-parallel over batch across 8 NeuronCores (2 batches/core).
Host-side prep (free): x is pre-transposed to [DIM, T] bf16 per batch, the
Wq/Wk projection weights are stacked so one matmul produces [Q^T; K^T], and
the rotary coefficient tables ship as one [128, T] f16 payload expanded
on-device. The pair-swap for rotary is a permutation matmul on-device.

Schedule: input loading is HBM-bound (~25us for 8MB), so each batch's
prologue is chained per 512-wide tile (proj -> psum copy -> pair-swap ->
rotary -> V transposes) and the first attention block for b0 is emitted
BEFORE b1's prologue — the ScalarE exp stream (the steady-state bottleneck,
~1.1us per [128,1024] tile) starts as soon as tile 0 of b0 is through.
b1's psum copies run on the DVE so the in-order ScalarE stream can't stall
the exps behind them. Per-batch psum rings (tag po<b>) keep the two
batches' pipelines independent; everything else is balanced across engines
(input DMAs on GpSimd+Sync queues, consts on the ScalarE HWDGE queue,
rotary dups on GpSimd, output scale-copies on DVE).

Per-core graph (per batch):
  QK^T = Wqk^T @ x^T (PE, bf16)            -> [128, T] psum
  swap = P_pairswap @ QK^T (PE, f16)
  q~/k~ = QK^T*fr + swap*fi (DVE, f16)     -> rotary applied, [64, T] each
  S^T[j-chunk] = k~[j].T @ q~ (PE, f16)    -> [128, 1024] psum per chunk
  P^T = exp(S^T / sqrt(512)) (ACT, fp16)   -> no max-subtraction needed:
        |S|/sqrt(512) <= ~5 so exp is safely bounded in fp16/f32
  out^T += V~[j].T @ P^T (PE, fp16)        -> V~ has a ones column so row 64
        accumulates the softmax denominator
  out^T * 2^-6 -> f16 (DVE), DMA to DRAM [65, T]; host divides row 64 and
        transposes back to [T, 64] f32.
"""

import os
import sys

for _p in ("/opt/trn_rl_repo", "/root/.axon_site/_ro/trn_rl_repo"):
    if os.path.isdir(_p) and _p not in sys.path:
        sys.path.append(_p)

import numpy as np
import ml_dtypes

import concourse.bass as bass
import concourse.mybir as mybir
import concourse.tile as tile
from concourse import bacc
from concourse.bass import ts
from concourse.bass_utils import run_bass_kernel_spmd

F32 = mybir.dt.float32
F32R = mybir.dt.float32r
BF16 = mybir.dt.bfloat16
F16 = mybir.dt.float16

B, T, DIM, HEAD = 16, 2048, 512, 64
NCORES = 8
BPC = B // NCORES          # batches per core
NCC = DIM // 128           # contraction chunks
NT = T // 512              # 512-wide tiles along t
NJ = T // 128              # key chunks
IH = T // 512              # query quarter blocks
IW = 512                   # query block width
OSCALE = 1.0 / 64.0        # keeps out^T accumulators in f16 range


def _build():
    scale = 1.0 / np.sqrt(np.float32(DIM))
    nc = bacc.Bacc(None, target_bir_lowering=False)
    xt_e = nc.declare_dram_parameter("xt", [BPC, DIM, T], BF16, isOutput=False)
    w_e = nc.declare_dram_parameter("w", [128, NCC * (128 + HEAD)], BF16,
                                    isOutput=False)
    frfi_e = nc.declare_dram_parameter("frfi", [128, T], F16, isOutput=False)
    id_e = nc.declare_dram_parameter("ident", [128, 128], F16, isOutput=False)
    out_e = nc.declare_dram_parameter("out", [BPC, HEAD + 1, T], F16,
                                      isOutput=True)

    with tile.TileContext(nc) as tc:
        with (
            tc.tile_pool(name="consts", bufs=1) as cp,
            tc.tile_pool(name="xt", bufs=1) as xp,
            tc.tile_pool(name="big", bufs=1) as bp,
            tc.tile_pool(name="vtp", bufs=1) as vp,
            tc.tile_pool(name="ptp", bufs=5) as pp,
            tc.tile_pool(name="op", bufs=3) as op,
            tc.tile_pool(name="psX", bufs=2, space="PSUM") as psX,
            tc.tile_pool(name="psS", bufs=2, space="PSUM") as psS,
        ):
            # Consts on the ScalarE HWDGE queue (empty at start) so they land
            # before the xt bulk transfers saturate the other two queues.
            w_all = cp.tile([128, NCC * (128 + HEAD)], BF16, tag="w")
            nc.scalar.dma_start(out=w_all, in_=w_e[:])
            wqk_t = [w_all[:, ci * 192:ci * 192 + 128] for ci in range(NCC)]
            wv_t = [w_all[:, ci * 192 + 128:ci * 192 + 192]
                    for ci in range(NCC)]
            frfi_t = cp.tile([128, T], F16, tag="frfi")
            nc.scalar.dma_start(out=frfi_t, in_=frfi_e[:])
            id16 = cp.tile([128, 128], F16, tag="id16")
            nc.scalar.dma_start(out=id16, in_=id_e[:])

            # Input DMAs: [128, T/2] half-chunks, b0 first, low halves of
            # all chunks before high halves, queues alternated — the first
            # projection starts after ~2MB instead of ~4MB, and b1's data
            # finishes sooner.
            HT = T // 2
            xts_b = []
            for b in range(BPC):
                halves = [[], []]
                for h in range(2):
                    for ci in range(NCC):
                        xh = xp.tile([128, HT], BF16, tag=f"xt{b}_{ci}_{h}")
                        eng = nc.gpsimd if ci % 2 == 0 else nc.sync
                        eng.dma_start(out=xh,
                                      in_=xt_e[b, ts(ci, 128), ts(h, HT)])
                        halves[h].append(xh)
                xts_b.append(halves)

            # fr stays f16 (multiplied with f16 qk), fi is f32 (multiplied
            # with the f32 psum swap output).
            fr_t = cp.tile([128, T], F16, tag="fr")
            nc.vector.tensor_copy(fr_t[0:64, :], frfi_t[0:64, :])
            nc.vector.tensor_copy(fr_t[64:128, :], frfi_t[0:64, :])
            fi_t = cp.tile([128, T], F16, tag="fi")
            nc.vector.tensor_copy(fi_t[0:64, :], frfi_t[64:128, :])
            nc.vector.tensor_copy(fi_t[64:128, :], frfi_t[64:128, :])

            per_b = [None] * BPC
            vts_b = [None] * BPC

            def prologue(b):
                """Per-tile chained prologue: tile tt is attention-ready
                before tile tt+1's projection finishes."""
                halves = xts_b[b]
                qk_s = bp.tile([128, T], F16, tag=f"qk{b}")
                vT_s = bp.tile([HEAD, T], F16, tag=f"vT{b}")
                qd = bp.tile([128, T], F16, tag=f"qd{b}")
                kd = bp.tile([128, T], F16, tag=f"kd{b}")
                t1 = bp.tile([128, T], F16, tag=f"t1{b}")
                t2 = bp.tile([128, T], F16, tag=f"t2{b}")
                vts = []
                # b0's psum copies go on ScalarE (idle before the exps);
                # b1's go on the DVE so the in-order ScalarE stream can't
                # block b0's exps behind b1's late psums.
                if b == 0:
                    copy = nc.scalar.copy
                else:
                    def copy(out, in_):
                        nc.vector.tensor_copy(out, in_)
                sw_s = bp.tile([128, T], F16, tag=f"sw{b}")
                swq = nc.sync if b == 0 else nc.gpsimd
                for tt in range(NT):
                    sl = ts(tt, 512)
                    xs = halves[tt // 2]
                    xsl = ts(tt % 2, 512)
                    pqk = psX.tile([128, 512], F32, tag=f"po{b}")
                    for ci in range(NCC):
                        nc.tensor.matmul(pqk, wqk_t[ci], xs[ci][:, xsl],
                                         start=(ci == 0), stop=(ci == NCC - 1))
                    copy(out=qk_s[:, sl], in_=pqk)
                    pv = psX.tile([HEAD, 512], F32, tag=f"po{b}")
                    for ci in range(NCC):
                        nc.tensor.matmul(pv, wv_t[ci], xs[ci][:, xsl],
                                         start=(ci == 0), stop=(ci == NCC - 1))
                    copy(out=vT_s[:, sl], in_=pv)

                    # Pair-swap via partition-strided SBUF-to-SBUF DMAs
                    # (replaces a PE permutation matmul).
                    swq.dma_start(out=sw_s[0:127:2, sl], in_=qk_s[1:128:2, sl])
                    swq.dma_start(out=sw_s[1:128:2, sl], in_=qk_s[0:127:2, sl])
                    nc.vector.tensor_tensor(out=t1[:, sl], in0=qk_s[:, sl],
                                            in1=fr_t[:, sl],
                                            op=mybir.AluOpType.mult)
                    nc.vector.tensor_tensor(out=t2[:, sl], in0=sw_s[:, sl],
                                            in1=fi_t[:, sl],
                                            op=mybir.AluOpType.mult)
                    nc.vector.tensor_tensor(out=qd[0:64, sl],
                                            in0=t1[0:64, sl],
                                            in1=t2[0:64, sl],
                                            op=mybir.AluOpType.add)
                    dupq = nc.gpsimd if b == 0 else nc.sync
                    dupq.dma_start(out=qd[64:128, sl], in_=qd[0:64, sl])
                    nc.vector.tensor_tensor(out=kd[64:128, sl],
                                            in0=t1[64:128, sl],
                                            in1=t2[64:128, sl],
                                            op=mybir.AluOpType.add)
                    dupq.dma_start(out=kd[0:64, sl], in_=kd[64:128, sl])

                    for j in range(4 * tt, 4 * tt + 4):
                        pvt = psX.tile([128, HEAD], F16, tag=f"po{b}")
                        nc.tensor.transpose(pvt, vT_s[:, ts(j, 128)],
                                            id16[0:HEAD, 0:HEAD])
                        vt = vp.tile([128, HEAD + 1], F16, tag=f"vt{b}_{j}")
                        nc.vector.tensor_copy(vt[:, 0:HEAD], pvt)
                        nc.vector.memset(vt[:, HEAD:HEAD + 1], 1.0)
                        vts.append(vt)
                per_b[b] = (qd, kd)
                vts_b[b] = vts

            NP = NJ // 2

            def attn(ih, jp, b, po_t):
                jA, jB = 2 * jp, 2 * jp + 1
                qd, kd = per_b[b]
                vts = vts_b[b]
                sp = psS.tile([128, 1024], F32, tag="s")
                nc.tensor.matmul(sp[:, 0:512],
                                 kd[0:64, ts(jA, 128)],
                                 qd[0:64, ts(ih, 512)],
                                 start=True, stop=True)
                nc.tensor.matmul(sp[:, 512:1024],
                                 kd[64:128, ts(jB, 128)],
                                 qd[64:128, ts(ih, 512)],
                                 start=True, stop=True)
                pT = pp.tile([128, 1024], F16, tag="pT")
                nc.scalar.activation(out=pT, in_=sp,
                                     func=mybir.ActivationFunctionType.Exp,
                                     scale=float(scale))
                nc.tensor.matmul(po_t, vts[jA], pT[:, 0:512],
                                 start=(jp == 0), stop=False,
                                 skip_group_check=True)
                nc.tensor.matmul(po_t, vts[jB], pT[:, 512:1024],
                                 start=False, stop=(jp == NP - 1),
                                 skip_group_check=True)

            def emit_out(ih, b, po_t):
                oc = op.tile([HEAD + 1, IW], F16, tag="oc")
                nc.vector.tensor_scalar_mul(out=oc, in0=po_t,
                                            scalar1=float(OSCALE))
                nc.sync.dma_start(out=out_e[b, :, ts(ih, IW)], in_=oc)

            # b0 prologue, then b0's first attention block BEFORE b1's
            # prologue: the exp stream starts while b1's inputs are still
            # in flight over HBM.
            prologue(0)
            po00 = psX.tile([HEAD + 1, IW], F32, tag="po0")
            for jp in range(NP):
                attn(0, jp, 0, po00)
            emit_out(0, 0, po00)

            prologue(1)
            po01 = psX.tile([HEAD + 1, IW], F32, tag="po1")
            for jp in range(NP):
                attn(0, jp, 1, po01)
            emit_out(0, 1, po01)

            # Remaining blocks: batches interleaved per key pair to keep the
            # ScalarE exp stream saturated.
            for ih in range(1, IH):
                pos = []
                for b in range(BPC):
                    po_t = psX.tile([HEAD + 1, IW], F32, tag=f"po{b}")
                    pos.append(po_t)
                for jp in range(NP):
                    for b in range(BPC):
                        attn(ih, jp, b, pos[b])
                for b in range(BPC):
                    emit_out(ih, b, pos[b])
    nc.compile()
    return nc


def _prep_consts(Wq, Wk, Wv, fx_real, fx_imag, fy_real, fy_imag):
    WqT = np.asarray(Wq, np.float32).T
    WkT = np.asarray(Wk, np.float32).T
    WvT = np.asarray(Wv, np.float32).T
    wqk = np.concatenate([WqT, WkT], axis=1).reshape(NCC, 128, 128)
    wv = WvT.reshape(NCC, 128, HEAD)
    w = np.concatenate([wqk, wv], axis=2)          # [NCC, 128, 192]
    w = np.ascontiguousarray(w.transpose(1, 0, 2).reshape(128, -1)).astype(
        ml_dtypes.bfloat16)

    fx_real = np.asarray(fx_real, np.float32)
    fx_imag = np.asarray(fx_imag, np.float32)
    fy_real = np.asarray(fy_real, np.float32)
    fy_imag = np.asarray(fy_imag, np.float32)
    fr64 = np.zeros((64, T), np.float32)
    fi64 = np.zeros((64, T), np.float32)
    for h in range(64):
        if h < 32:
            frs, fis, p = fx_real, fx_imag, h // 2
        else:
            frs, fis, p = fy_real, fy_imag, (h - 32) // 2
        fr64[h] = frs[:, p]
        fi64[h] = fis[:, p] * (-1.0 if h % 2 == 0 else 1.0)
    frfi = np.concatenate([fr64, fi64], axis=0).astype(np.float16)
    ident = np.eye(128, dtype=np.float16)
    return dict(w=w, frfi=frfi, ident=ident)


_NC_CACHE = {}


def _get_nc():
    if "nc" not in _NC_CACHE:
        _NC_CACHE["nc"] = _build()
    return _NC_CACHE["nc"]


def kernel(x, Wq, Wk, Wv, fx_real, fx_imag, fy_real, fy_imag):
    x = np.asarray(x, np.float32)
    xt = np.ascontiguousarray(x.transpose(0, 2, 1)).astype(ml_dtypes.bfloat16)
    consts = _prep_consts(Wq, Wk, Wv, fx_real, fx_imag, fy_real, fy_imag)
    in_maps = []
    for c in range(NCORES):
        m = {"xt": xt[c * BPC:(c + 1) * BPC]}
        m.update(consts)
        in_maps.append(m)
    nc = _get_nc()
    res = run_bass_kernel_spmd(nc, in_maps, core_ids=list(range(NCORES)))
    # res: per-core [BPC, 65, T] f16 -> divide by denominator row, transpose
    outs = []
    for c in range(NCORES):
        o = np.asarray(res.results[c]["out"], np.float32)
        outs.append((o[:, 0:HEAD, :] / o[:, HEAD:HEAD + 1, :]).transpose(0, 2, 1))
    return np.ascontiguousarray(np.concatenate(outs, axis=0))
